# revision 32
# baseline (speedup 1.0000x reference)
"""Multi-head masked attention on 8 TRN2 NeuronCores.

Sharding: data-parallel over batch. B=8 -> one batch element per core,
no collectives. Each core computes the full 8-head attention + output
projection for its batch element.

Per-core algorithm (all matmuls bf16, PSUM accumulation f32):
  xT   = x^T                       (PE transpose, [d, n] layout)
  qT_h = Wq_h^T @ x^T  [64, 1024]  (lhsT = Wq pair, rhs = xT)
  kT_h = Wk_h^T @ x^T  [64, 1024]
  v_h  = x @ Wv_h      [1024, 64]  (lhsT = xT, rhs = Wv pair), augmented
         with a ones column -> v_aug [m, 65]
  S^T  = kT^T qT       [m, n]      per 128-row m-tile
  P    = exp(S^T/8) * keepT        (ACT exp w/ scale, DVE mask multiply;
                                    no max-subtraction needed: |S/8| small,
                                    masked entries zeroed via keep=1-mask)
  hT   = v_aug^T @ P   [65, n]     row 64 = softmax denominator
  hT_n = hT[0:64] * (1/denom)      (DVE recip + DMA partition-broadcast)
  out  = sum_h hT_h^T @ Wo_h       (accumulated over heads in PSUM)
"""

import sys

for _p in ("/opt/trn_rl_repo", "/root/.axon_site/_ro/trn_rl_repo"):
    if _p not in sys.path:
        sys.path.insert(0, _p)

from contextlib import ExitStack

import numpy as np

import concourse.bass as bass
import concourse.bacc as bacc
import concourse.mybir as mybir
from concourse.bass_utils import run_bass_kernel_spmd
from concourse.masks import make_identity
from concourse.tile import TileContext

dt = mybir.dt
AF = mybir.ActivationFunctionType

B = 8
N = 1024
D = 512
H = 8
DK = 64
P = 128
NT = N // P  # 8 n-tiles (also m-tiles)
DC = D // P  # 4 d-chunks
HP = H // 2  # 4 head pairs


def build_bass(debug=False):
    nc = bacc.Bacc()

    x_d = nc.declare_dram_parameter("x", [N, D], dt.float32, isOutput=False)
    m_d = nc.declare_dram_parameter("mask", [N, N], dt.uint8, isOutput=False)
    wq_d = nc.declare_dram_parameter("wq", [H, D, DK], dt.float32, isOutput=False)
    wk_d = nc.declare_dram_parameter("wk", [H, D, DK], dt.float32, isOutput=False)
    wv_d = nc.declare_dram_parameter("wv", [H, D, DK], dt.float32, isOutput=False)
    wo_d = nc.declare_dram_parameter("wo", [H, DK, D], dt.float32, isOutput=False)
    o_d = nc.declare_dram_parameter("out", [N, D], dt.float32, isOutput=True)
    dbg = {}
    if debug:
        for nm, shp in (
            ("dbg_xT", [P, DC * N]),
            ("dbg_keepT", [P, NT * N]),
            ("dbg_qT", [P, HP * N]),
            ("dbg_kT", [P, HP * N]),
            ("dbg_v", [P, NT * H * (DK + 1)]),
            ("dbg_hT", [DK, H * N]),
            ("dbg_p00", [P, N]),
        ):
            dbg[nm] = nc.declare_dram_parameter(nm, shp, dt.bfloat16, isOutput=True)

    with TileContext(nc) as tc, ExitStack() as ctx:
        persist = ctx.enter_context(tc.tile_pool(name="persist", bufs=1))
        stage = ctx.enter_context(tc.tile_pool(name="stage", bufs=1))
        stage_w = ctx.enter_context(tc.tile_pool(name="stage_w", bufs=2))
        expp = ctx.enter_context(tc.tile_pool(name="expp", bufs=2))
        pp = ctx.enter_context(tc.tile_pool(name="pp", bufs=3))
        recp = ctx.enter_context(tc.tile_pool(name="recp", bufs=1))
        dramp = ctx.enter_context(tc.tile_pool(name="dramp", bufs=2, space="DRAM"))
        ps_sh = ctx.enter_context(tc.tile_pool(name="ps_sh", bufs=2, space="PSUM"))
        ps_ht = ctx.enter_context(tc.tile_pool(name="ps_ht", bufs=2, space="PSUM"))

        # ---- identity for PE transposes (via regular matmul) ----
        identbf = persist.tile([P, P], dt.bfloat16)
        make_identity(nc, identbf)

        # ---- load inputs ----
        x_f32 = stage.tile([P, NT, D], dt.float32)
        nc.sync.dma_start(out=x_f32, in_=x_d[:].rearrange("(i p) d -> p i d", p=P))

        # weight layout: [P=d%128, DC=d//128, H*DK] -> a (head-pair, d-chunk)
        # stationary slice [:, j, hp*128:(hp+1)*128] is one contiguous free dim
        mask_u8 = stage.tile([P, NT, N], dt.uint8)
        nc.sync.dma_start(out=mask_u8, in_=m_d[:].rearrange("(i p) m -> p i m", p=P))

        # ---- weights: DMA f32 chunks through small staging, convert to bf16
        wq_bf = persist.tile([P, DC, H * DK], dt.bfloat16)
        wk_bf = persist.tile([P, DC, H * DK], dt.bfloat16)
        wv_bf = persist.tile([P, DC, H * DK], dt.bfloat16)
        for w_bf, w_d in ((wq_bf, wq_d), (wk_bf, wk_d), (wv_bf, wv_d)):
            src = w_d[:].rearrange("h (j p) k -> j p h k", p=P)
            for j in range(DC):
                wstg = stage_w.tile([P, H, DK], dt.float32, tag="wstg")
                nc.sync.dma_start(out=wstg, in_=src[j])
                nc.vector.tensor_copy(
                    out=w_bf[:, j, :], in_=wstg.rearrange("p h k -> p (h k)")
                )
        wo_bf = persist.tile([DK, H, D], dt.bfloat16)
        wo_src = wo_d[:].rearrange("h v d -> v h d")
        for c in range(4):
            wstg2 = stage_w.tile([DK, 2, D], dt.float32, tag="wstg2")
            nc.sync.dma_start(out=wstg2, in_=wo_src[:, 2 * c : 2 * c + 2, :])
            nc.vector.tensor_copy(out=wo_bf[:, 2 * c : 2 * c + 2, :], in_=wstg2)

        # ---- xT = x^T ----
        # Transposes are regular matmuls (lhsT=block, rhs=I): the is_transpose
        # lowering (S3_LW) only supports a single sync-wait and walrus rejects
        # Tile's two-wait instructions.
        x_bf = stage.tile([P, NT, D], dt.bfloat16)
        nc.vector.tensor_copy(out=x_bf, in_=x_f32)
        xT = persist.tile([P, DC, N], dt.bfloat16)
        for j in range(DC):
            for half in range(2):
                ps = ps_sh.tile([P, N], dt.float32, tag="ps_sh")
                for k in range(4):
                    ni = half * 4 + k
                    nc.tensor.matmul(
                        ps[:, k * P : (k + 1) * P],
                        lhsT=x_bf[:, ni, j * P : (j + 1) * P],
                        rhs=identbf,
                        start=True,
                        stop=True,
                    )
                nc.vector.tensor_copy(
                    out=xT[:, j, half * 512 : (half + 1) * 512], in_=ps[:, 0:512]
                )

        # ---- keep = 1 - mask (bf16), then keepT via PE transpose ----
        m_bf = stage.tile([P, NT, N], dt.bfloat16)
        nc.vector.tensor_copy(out=m_bf, in_=mask_u8)
        keep_bf = stage.tile([P, NT, N], dt.bfloat16)
        nc.vector.tensor_scalar(
            out=keep_bf,
            in0=m_bf,
            scalar1=-1.0,
            scalar2=1.0,
            op0=mybir.AluOpType.mult,
            op1=mybir.AluOpType.add,
        )
        keepT = persist.tile([P, NT, N], dt.bfloat16)
        for mi in range(NT):
            for half in range(2):
                ps = ps_sh.tile([P, N], dt.float32, tag="ps_sh")
                for k in range(4):
                    ni = half * 4 + k
                    nc.tensor.matmul(
                        ps[:, k * P : (k + 1) * P],
                        lhsT=keep_bf[:, ni, mi * P : (mi + 1) * P],
                        rhs=identbf,
                        start=True,
                        stop=True,
                    )
                nc.vector.tensor_copy(
                    out=keepT[:, mi, half * 512 : (half + 1) * 512], in_=ps[:, 0:512]
                )

        # ---- projections ----
        qT = persist.tile([P, HP, N], dt.bfloat16)
        kT = persist.tile([P, HP, N], dt.bfloat16)
        for dst, w in ((qT, wq_bf), (kT, wk_bf)):
            for hp in range(HP):
                for c in range(2):
                    ps = ps_sh.tile([P, N], dt.float32, tag="ps_sh")
                    for j in range(DC):
                        nc.tensor.matmul(
                            ps[:, c * 512 : (c + 1) * 512],
                            lhsT=w[:, j, hp * P : (hp + 1) * P],
                            rhs=xT[:, j, c * 512 : (c + 1) * 512],
                            start=(j == 0),
                            stop=(j == DC - 1),
                        )
                    nc.vector.tensor_copy(
                        out=dst[:, hp, c * 512 : (c + 1) * 512],
                        in_=ps[:, c * 512 : (c + 1) * 512],
                    )

        # v_aug: [m-part, m-tile, head, 65]; col 64 = ones (softmax denom trick)
        v_sb = persist.tile([P, NT, H, DK + 1], dt.bfloat16)
        nc.vector.memset(v_sb[:, :, :, DK : DK + 1], 1.0)
        for i in range(NT):
            ps = ps_sh.tile([P, N], dt.float32, tag="ps_sh")
            for j in range(DC):
                # one accumulation group over the full 512-col bank: PSUM
                # start=True zeroes the whole bank, so groups must not
                # interleave within a bank
                nc.tensor.matmul(
                    ps[:, 0:512],
                    lhsT=xT[:, j, i * P : (i + 1) * P],
                    rhs=wv_bf[:, j, :],
                    start=(j == 0),
                    stop=(j == DC - 1),
                )
            nc.vector.tensor_copy(
                out=v_sb[:, i, :, 0:DK],
                in_=ps[:, 0:512].rearrange("p (h k) -> p h k", k=DK),
            )

        # ---- attention per head ----
        hT = persist.tile([DK, H, N], dt.bfloat16)
        for h in range(H):
            hp, r0 = h // 2, (h % 2) * DK
            q_h = qT[r0 : r0 + DK, hp, :]
            k_h = kT[r0 : r0 + DK, hp, :]

            ps_h = ps_ht.tile([DK + 1, N], dt.float32, tag="ps_ht")
            for mi in range(NT):
                ps_s = ps_sh.tile([P, N], dt.float32, tag="ps_sh")
                for c in range(2):
                    nc.tensor.matmul(
                        ps_s[:, c * 512 : (c + 1) * 512],
                        lhsT=k_h[:, mi * P : (mi + 1) * P],
                        rhs=q_h[:, c * 512 : (c + 1) * 512],
                        start=True,
                        stop=True,
                    )
                e_t = expp.tile([P, N], dt.bfloat16, tag="e")
                nc.scalar.activation(out=e_t, in_=ps_s, func=AF.Exp, scale=0.125)
                p_t = pp.tile([P, N], dt.bfloat16, tag="p")
                nc.vector.tensor_mul(p_t, e_t, keepT[:, mi, :])
                if debug and h == 0 and mi == 0:
                    nc.sync.dma_start(out=dbg["dbg_p00"][:], in_=p_t)
                for c in range(2):
                    nc.tensor.matmul(
                        ps_h[:, c * 512 : (c + 1) * 512],
                        lhsT=v_sb[:, mi, h, :],
                        rhs=p_t[:, c * 512 : (c + 1) * 512],
                        start=(mi == 0),
                        stop=(mi == NT - 1),
                    )

            # normalize: rows 0:64 / row 64
            rec_row = recp.tile([1, N], dt.float32, tag="rrow")
            nc.vector.reciprocal(out=rec_row, in_=ps_h[DK : DK + 1, :])
            rec_dram = dramp.tile([1, N], dt.float32, tag="rdram")
            nc.sync.dma_start(out=rec_dram, in_=rec_row)
            rec64 = recp.tile([DK, N], dt.float32, tag="r64")
            nc.sync.dma_start(out=rec64, in_=rec_dram.to_broadcast((DK, N)))
            nc.vector.tensor_mul(hT[:, h, :], ps_h[0:DK, :], rec64)

        # ---- output projection: out[n, d] = sum_h hT_h^T @ Wo_h ----
        out_sb = persist.tile([P, NT, D], dt.float32)
        for ni in range(NT):
            ps = ps_sh.tile([P, N], dt.float32, tag="ps_sh")
            for h in range(H):
                nc.tensor.matmul(
                    ps[:, 0:512],
                    lhsT=hT[:, h, ni * P : (ni + 1) * P],
                    rhs=wo_bf[:, h, :],
                    start=(h == 0),
                    stop=(h == H - 1),
                )
            nc.vector.tensor_copy(out=out_sb[:, ni, :], in_=ps[:, 0:512])

        nc.sync.dma_start(
            out=o_d[:].rearrange("(i p) d -> p i d", p=P), in_=out_sb
        )

        if debug:
            for nm, t, pat in (
                ("dbg_xT", xT, "p a b -> p (a b)"),
                ("dbg_keepT", keepT, "p a b -> p (a b)"),
                ("dbg_qT", qT, "p a b -> p (a b)"),
                ("dbg_kT", kT, "p a b -> p (a b)"),
                ("dbg_v", v_sb, "p a b c -> p (a b c)"),
                ("dbg_hT", hT, "p a b -> p (a b)"),
            ):
                nc.sync.dma_start(out=dbg[nm][:], in_=t.rearrange(pat))

    nc.finalize()
    return nc


_NC_CACHE = None


def kernel(**inputs: np.ndarray) -> np.ndarray:
    global _NC_CACHE
    x = inputs["x"]
    mask = inputs["mask"]
    Wq, Wk, Wv, Wo = inputs["Wq"], inputs["Wk"], inputs["Wv"], inputs["Wo"]

    if _NC_CACHE is None:
        _NC_CACHE = build_bass()
    nc = _NC_CACHE

    in_maps = []
    for b in range(B):
        in_maps.append(
            {
                "x": np.ascontiguousarray(x[b], dtype=np.float32),
                "mask": np.ascontiguousarray(mask[b]).astype(np.uint8),
                "wq": np.ascontiguousarray(Wq, dtype=np.float32),
                "wk": np.ascontiguousarray(Wk, dtype=np.float32),
                "wv": np.ascontiguousarray(Wv, dtype=np.float32),
                "wo": np.ascontiguousarray(Wo, dtype=np.float32),
            }
        )

    res = run_bass_kernel_spmd(nc, in_maps, core_ids=list(range(B)))
    out = np.stack([np.asarray(res.results[b]["out"]) for b in range(B)], axis=0)
    return out.astype(np.float32)


if __name__ == "__main__":
    rng = np.random.default_rng(0)
    ins = {
        "x": rng.standard_normal((B, N, D), dtype=np.float32),
        "mask": rng.integers(0, 2, (B, N, N)).astype(bool),
        "Wq": (rng.standard_normal((H, D, DK)) * 0.001).astype(np.float32),
        "Wk": (rng.standard_normal((H, D, DK)) * 0.001).astype(np.float32),
        "Wv": (rng.standard_normal((H, D, DK)) * 0.001).astype(np.float32),
        "Wo": (rng.standard_normal((H, DK, D)) * 0.001).astype(np.float32),
    }
    o = kernel(**ins)
    print(o.shape, o.dtype, np.abs(o).mean())


# revision 35
# speedup vs baseline: 1.0407x; 1.0407x over previous
"""Multi-head masked attention on 8 TRN2 NeuronCores.

Sharding: data-parallel over batch. B=8 -> one batch element per core,
no collectives. Each core computes the full 8-head attention + output
projection for its batch element.

Per-core algorithm (all matmuls bf16, PSUM accumulation f32):
  xT   = x^T                       (PE transpose, [d, n] layout)
  qT_h = Wq_h^T @ x^T  [64, 1024]  (lhsT = Wq pair, rhs = xT)
  kT_h = Wk_h^T @ x^T  [64, 1024]
  v_h  = x @ Wv_h      [1024, 64]  (lhsT = xT, rhs = Wv pair), augmented
         with a ones column -> v_aug [m, 65]
  S^T  = kT^T qT       [m, n]      per 128-row m-tile
  P    = exp(S^T/8) * keepT        (ACT exp w/ scale, DVE mask multiply;
                                    no max-subtraction needed: |S/8| small,
                                    masked entries zeroed via keep=1-mask)
  hT   = v_aug^T @ P   [65, n]     row 64 = softmax denominator
  hT_n = hT[0:64] * (1/denom)      (DVE recip + DMA partition-broadcast)
  out  = sum_h hT_h^T @ Wo_h       (accumulated over heads in PSUM)
"""

import sys

for _p in ("/opt/trn_rl_repo", "/root/.axon_site/_ro/trn_rl_repo"):
    if _p not in sys.path:
        sys.path.insert(0, _p)

from contextlib import ExitStack

import numpy as np

import concourse.bass as bass
import concourse.bacc as bacc
import concourse.mybir as mybir
from concourse.bass_utils import run_bass_kernel_spmd
from concourse.masks import make_identity
from concourse.tile import TileContext

dt = mybir.dt
AF = mybir.ActivationFunctionType

B = 8
N = 1024
D = 512
H = 8
DK = 64
P = 128
NT = N // P  # 8 n-tiles (also m-tiles)
DC = D // P  # 4 d-chunks
HP = H // 2  # 4 head pairs


def build_bass(debug=False):
    nc = bacc.Bacc()

    x_d = nc.declare_dram_parameter("x", [N, D], dt.float32, isOutput=False)
    m_d = nc.declare_dram_parameter("mask", [N, N], dt.uint8, isOutput=False)
    wq_d = nc.declare_dram_parameter("wq", [H, D, DK], dt.float32, isOutput=False)
    wk_d = nc.declare_dram_parameter("wk", [H, D, DK], dt.float32, isOutput=False)
    wv_d = nc.declare_dram_parameter("wv", [H, D, DK], dt.float32, isOutput=False)
    wo_d = nc.declare_dram_parameter("wo", [H, DK, D], dt.float32, isOutput=False)
    o_d = nc.declare_dram_parameter("out", [N, D], dt.float32, isOutput=True)
    dbg = {}
    if debug:
        for nm, shp in (
            ("dbg_xT", [P, DC * N]),
            ("dbg_keepT", [P, NT * N]),
            ("dbg_qT", [P, HP * N]),
            ("dbg_kT", [P, HP * N]),
            ("dbg_v", [P, NT * H * (DK + 1)]),
            ("dbg_hT", [DK, H * N]),
            ("dbg_p00", [P, N]),
        ):
            dbg[nm] = nc.declare_dram_parameter(nm, shp, dt.bfloat16, isOutput=True)

    with TileContext(nc) as tc, ExitStack() as ctx:
        persist = ctx.enter_context(tc.tile_pool(name="persist", bufs=1))
        stage = ctx.enter_context(tc.tile_pool(name="stage", bufs=1))
        stage_w = ctx.enter_context(tc.tile_pool(name="stage_w", bufs=2))
        expp = ctx.enter_context(tc.tile_pool(name="expp", bufs=2))
        pp = ctx.enter_context(tc.tile_pool(name="pp", bufs=3))
        recp = ctx.enter_context(tc.tile_pool(name="recp", bufs=1))
        dramp = ctx.enter_context(tc.tile_pool(name="dramp", bufs=2, space="DRAM"))
        ps_sh = ctx.enter_context(tc.tile_pool(name="ps_sh", bufs=2, space="PSUM"))
        ps_ht = ctx.enter_context(tc.tile_pool(name="ps_ht", bufs=2, space="PSUM"))

        # ---- identity for PE transposes (via regular matmul) ----
        identbf = persist.tile([P, P], dt.bfloat16)
        make_identity(nc, identbf)

        # ---- load inputs ----
        x_f32 = stage.tile([P, NT, D], dt.float32)
        nc.sync.dma_start(out=x_f32, in_=x_d[:].rearrange("(i p) d -> p i d", p=P))

        # weight layout: [P=d%128, DC=d//128, H*DK] -> a (head-pair, d-chunk)
        # stationary slice [:, j, hp*128:(hp+1)*128] is one contiguous free dim
        mask_u8 = stage.tile([P, NT, N], dt.uint8)
        nc.sync.dma_start(out=mask_u8, in_=m_d[:].rearrange("(i p) m -> p i m", p=P))

        # ---- weights: DMA f32 chunks through small staging, convert to bf16
        wq_bf = persist.tile([P, DC, H * DK], dt.bfloat16)
        wk_bf = persist.tile([P, DC, H * DK], dt.bfloat16)
        wv_bf = persist.tile([P, DC, H * DK], dt.bfloat16)
        for w_bf, w_d in ((wq_bf, wq_d), (wk_bf, wk_d), (wv_bf, wv_d)):
            src = w_d[:].rearrange("h (j p) k -> j p h k", p=P)
            for j in range(DC):
                wstg = stage_w.tile([P, H, DK], dt.float32, tag="wstg")
                nc.sync.dma_start(out=wstg, in_=src[j])
                nc.vector.tensor_copy(
                    out=w_bf[:, j, :], in_=wstg.rearrange("p h k -> p (h k)")
                )
        wo_bf = persist.tile([DK, H, D], dt.bfloat16)
        wo_src = wo_d[:].rearrange("h v d -> v h d")
        for c in range(4):
            wstg2 = stage_w.tile([DK, 2, D], dt.float32, tag="wstg2")
            nc.sync.dma_start(out=wstg2, in_=wo_src[:, 2 * c : 2 * c + 2, :])
            nc.vector.tensor_copy(out=wo_bf[:, 2 * c : 2 * c + 2, :], in_=wstg2)

        # ---- xT = x^T ----
        # Transposes are regular matmuls (lhsT=block, rhs=I): the is_transpose
        # lowering (S3_LW) only supports a single sync-wait and walrus rejects
        # Tile's two-wait instructions.
        x_bf = stage.tile([P, NT, D], dt.bfloat16)
        nc.vector.tensor_copy(out=x_bf, in_=x_f32)
        xT = persist.tile([P, DC, N], dt.bfloat16)
        for j in range(DC):
            for half in range(2):
                ps = ps_sh.tile([P, N], dt.float32, tag="ps_sh")
                for k in range(4):
                    ni = half * 4 + k
                    nc.tensor.matmul(
                        ps[:, k * P : (k + 1) * P],
                        lhsT=x_bf[:, ni, j * P : (j + 1) * P],
                        rhs=identbf,
                        start=True,
                        stop=True,
                    )
                nc.vector.tensor_copy(
                    out=xT[:, j, half * 512 : (half + 1) * 512], in_=ps[:, 0:512]
                )

        # ---- keep = 1 - mask (bf16), then keepT via PE transpose ----
        m_bf = stage.tile([P, NT, N], dt.bfloat16)
        nc.gpsimd.tensor_copy(out=m_bf, in_=mask_u8)
        keep_bf = stage.tile([P, NT, N], dt.bfloat16)
        nc.gpsimd.tensor_scalar(
            out=keep_bf,
            in0=m_bf,
            scalar1=-1.0,
            scalar2=1.0,
            op0=mybir.AluOpType.mult,
            op1=mybir.AluOpType.add,
        )
        keepT = persist.tile([P, NT, N], dt.bfloat16)
        for mi in range(NT):
            for half in range(2):
                ps = ps_sh.tile([P, N], dt.float32, tag="ps_sh")
                for k in range(4):
                    ni = half * 4 + k
                    nc.tensor.matmul(
                        ps[:, k * P : (k + 1) * P],
                        lhsT=keep_bf[:, ni, mi * P : (mi + 1) * P],
                        rhs=identbf,
                        start=True,
                        stop=True,
                    )
                nc.scalar.activation(
                    out=keepT[:, mi, half * 512 : (half + 1) * 512],
                    in_=ps[:, 0:512],
                    func=AF.Copy,
                )

        # ---- projections ----
        qT = persist.tile([P, HP, N], dt.bfloat16)
        kT = persist.tile([P, HP, N], dt.bfloat16)
        for dst, w in ((qT, wq_bf), (kT, wk_bf)):
            for hp in range(HP):
                for c in range(2):
                    ps = ps_sh.tile([P, N], dt.float32, tag="ps_sh")
                    for j in range(DC):
                        nc.tensor.matmul(
                            ps[:, c * 512 : (c + 1) * 512],
                            lhsT=w[:, j, hp * P : (hp + 1) * P],
                            rhs=xT[:, j, c * 512 : (c + 1) * 512],
                            start=(j == 0),
                            stop=(j == DC - 1),
                        )
                    nc.scalar.activation(
                        out=dst[:, hp, c * 512 : (c + 1) * 512],
                        in_=ps[:, c * 512 : (c + 1) * 512],
                        func=AF.Copy,
                    )

        # v_aug: [m-part, m-tile, head, 65]; col 64 = ones (softmax denom trick)
        v_sb = persist.tile([P, NT, H, DK + 1], dt.bfloat16)
        nc.vector.memset(v_sb[:, :, :, DK : DK + 1], 1.0)
        for i in range(NT):
            ps = ps_sh.tile([P, N], dt.float32, tag="ps_sh")
            for j in range(DC):
                # one accumulation group over the full 512-col bank: PSUM
                # start=True zeroes the whole bank, so groups must not
                # interleave within a bank
                nc.tensor.matmul(
                    ps[:, 0:512],
                    lhsT=xT[:, j, i * P : (i + 1) * P],
                    rhs=wv_bf[:, j, :],
                    start=(j == 0),
                    stop=(j == DC - 1),
                )
            nc.scalar.activation(
                out=v_sb[:, i, :, 0:DK],
                in_=ps[:, 0:512].rearrange("p (h k) -> p h k", k=DK),
                func=AF.Copy,
            )

        # ---- attention per head ----
        hT = persist.tile([DK, H, N], dt.bfloat16)
        for h in range(H):
            hp, r0 = h // 2, (h % 2) * DK
            q_h = qT[r0 : r0 + DK, hp, :]
            k_h = kT[r0 : r0 + DK, hp, :]

            ps_h = ps_ht.tile([DK + 1, N], dt.float32, tag="ps_ht")
            for mi in range(NT):
                ps_s = ps_sh.tile([P, N], dt.float32, tag="ps_sh")
                for c in range(2):
                    nc.tensor.matmul(
                        ps_s[:, c * 512 : (c + 1) * 512],
                        lhsT=k_h[:, mi * P : (mi + 1) * P],
                        rhs=q_h[:, c * 512 : (c + 1) * 512],
                        start=True,
                        stop=True,
                    )
                e_t = expp.tile([P, N], dt.bfloat16, tag="e")
                nc.scalar.activation(out=e_t, in_=ps_s, func=AF.Exp, scale=0.125)
                p_t = pp.tile([P, N], dt.bfloat16, tag="p")
                nc.vector.tensor_mul(p_t, e_t, keepT[:, mi, :])
                if debug and h == 0 and mi == 0:
                    nc.sync.dma_start(out=dbg["dbg_p00"][:], in_=p_t)
                for c in range(2):
                    nc.tensor.matmul(
                        ps_h[:, c * 512 : (c + 1) * 512],
                        lhsT=v_sb[:, mi, h, :],
                        rhs=p_t[:, c * 512 : (c + 1) * 512],
                        start=(mi == 0),
                        stop=(mi == NT - 1),
                    )

            # normalize: rows 0:64 / row 64
            # denom row -> SBUF (ACT) -> DRAM -> partition-broadcast to 64
            # rows, then reciprocal on SBUF (neither reciprocal_approx_fast
            # nor DMA can read PSUM)
            den_row = recp.tile([1, N], dt.float32, tag="drow")
            nc.scalar.activation(out=den_row, in_=ps_h[DK : DK + 1, :], func=AF.Copy)
            den_dram = dramp.tile([1, N], dt.float32, tag="rdram")
            nc.sync.dma_start(out=den_dram, in_=den_row)
            den64 = recp.tile([DK, N], dt.float32, tag="d64")
            nc.sync.dma_start(out=den64, in_=den_dram.to_broadcast((DK, N)))
            rec64 = recp.tile([DK, N], dt.float32, tag="r64")
            nc.vector.reciprocal_approx_fast(out=rec64, in_=den64)
            nc.vector.tensor_mul(hT[:, h, :], ps_h[0:DK, :], rec64)

        # ---- output projection: out[n, d] = sum_h hT_h^T @ Wo_h ----
        out_sb = persist.tile([P, NT, D], dt.float32)
        for ni in range(NT):
            ps = ps_sh.tile([P, N], dt.float32, tag="ps_sh")
            for h in range(H):
                nc.tensor.matmul(
                    ps[:, 0:512],
                    lhsT=hT[:, h, ni * P : (ni + 1) * P],
                    rhs=wo_bf[:, h, :],
                    start=(h == 0),
                    stop=(h == H - 1),
                )
            nc.scalar.activation(out=out_sb[:, ni, :], in_=ps[:, 0:512], func=AF.Copy)

        nc.sync.dma_start(
            out=o_d[:].rearrange("(i p) d -> p i d", p=P), in_=out_sb
        )

        if debug:
            for nm, t, pat in (
                ("dbg_xT", xT, "p a b -> p (a b)"),
                ("dbg_keepT", keepT, "p a b -> p (a b)"),
                ("dbg_qT", qT, "p a b -> p (a b)"),
                ("dbg_kT", kT, "p a b -> p (a b)"),
                ("dbg_v", v_sb, "p a b c -> p (a b c)"),
                ("dbg_hT", hT, "p a b -> p (a b)"),
            ):
                nc.sync.dma_start(out=dbg[nm][:], in_=t.rearrange(pat))

    nc.finalize()
    return nc


_NC_CACHE = None


def kernel(**inputs: np.ndarray) -> np.ndarray:
    global _NC_CACHE
    x = inputs["x"]
    mask = inputs["mask"]
    Wq, Wk, Wv, Wo = inputs["Wq"], inputs["Wk"], inputs["Wv"], inputs["Wo"]

    if _NC_CACHE is None:
        _NC_CACHE = build_bass()
    nc = _NC_CACHE

    in_maps = []
    for b in range(B):
        in_maps.append(
            {
                "x": np.ascontiguousarray(x[b], dtype=np.float32),
                "mask": np.ascontiguousarray(mask[b]).astype(np.uint8),
                "wq": np.ascontiguousarray(Wq, dtype=np.float32),
                "wk": np.ascontiguousarray(Wk, dtype=np.float32),
                "wv": np.ascontiguousarray(Wv, dtype=np.float32),
                "wo": np.ascontiguousarray(Wo, dtype=np.float32),
            }
        )

    res = run_bass_kernel_spmd(nc, in_maps, core_ids=list(range(B)))
    out = np.stack([np.asarray(res.results[b]["out"]) for b in range(B)], axis=0)
    return out.astype(np.float32)


if __name__ == "__main__":
    rng = np.random.default_rng(0)
    ins = {
        "x": rng.standard_normal((B, N, D), dtype=np.float32),
        "mask": rng.integers(0, 2, (B, N, N)).astype(bool),
        "Wq": (rng.standard_normal((H, D, DK)) * 0.001).astype(np.float32),
        "Wk": (rng.standard_normal((H, D, DK)) * 0.001).astype(np.float32),
        "Wv": (rng.standard_normal((H, D, DK)) * 0.001).astype(np.float32),
        "Wo": (rng.standard_normal((H, DK, D)) * 0.001).astype(np.float32),
    }
    o = kernel(**ins)
    print(o.shape, o.dtype, np.abs(o).mean())


# revision 38
# speedup vs baseline: 1.0976x; 1.0547x over previous
"""Multi-head masked attention on 8 TRN2 NeuronCores.

Sharding: data-parallel over batch. B=8 -> one batch element per core,
no collectives. Each core computes the full 8-head attention + output
projection for its batch element.

Per-core algorithm (all matmuls bf16, PSUM accumulation f32):
  xT   = x^T                       (PE transpose, [d, n] layout)
  qT_h = Wq_h^T @ x^T  [64, 1024]  (lhsT = Wq pair, rhs = xT)
  kT_h = Wk_h^T @ x^T  [64, 1024]
  v_h  = x @ Wv_h      [1024, 64]  (lhsT = xT, rhs = Wv pair), augmented
         with a ones column -> v_aug [m, 65]
  S^T  = kT^T qT       [m, n]      per 128-row m-tile
  P    = exp(S^T/8) * keepT        (ACT exp w/ scale, DVE mask multiply;
                                    no max-subtraction needed: |S/8| small,
                                    masked entries zeroed via keep=1-mask)
  hT   = v_aug^T @ P   [65, n]     row 64 = softmax denominator
  hT_n = hT[0:64] * (1/denom)      (DVE recip + DMA partition-broadcast)
  out  = sum_h hT_h^T @ Wo_h       (accumulated over heads in PSUM)
"""

import sys

for _p in ("/opt/trn_rl_repo", "/root/.axon_site/_ro/trn_rl_repo"):
    if _p not in sys.path:
        sys.path.insert(0, _p)

from contextlib import ExitStack

import numpy as np

import concourse.bass as bass
import concourse.bacc as bacc
import concourse.mybir as mybir
from concourse.bass_utils import run_bass_kernel_spmd
from concourse.masks import make_identity
from concourse.tile import TileContext

dt = mybir.dt
AF = mybir.ActivationFunctionType

B = 8
N = 1024
D = 512
H = 8
DK = 64
P = 128
NT = N // P  # 8 n-tiles (also m-tiles)
DC = D // P  # 4 d-chunks
HP = H // 2  # 4 head pairs


def build_bass(debug=False):
    nc = bacc.Bacc()

    x_d = nc.declare_dram_parameter("x", [N, D], dt.float32, isOutput=False)
    m_d = nc.declare_dram_parameter("mask", [N, N], dt.uint8, isOutput=False)
    wq_d = nc.declare_dram_parameter("wq", [H, D, DK], dt.float32, isOutput=False)
    wk_d = nc.declare_dram_parameter("wk", [H, D, DK], dt.float32, isOutput=False)
    wv_d = nc.declare_dram_parameter("wv", [H, D, DK], dt.float32, isOutput=False)
    wo_d = nc.declare_dram_parameter("wo", [H, DK, D], dt.float32, isOutput=False)
    o_d = nc.declare_dram_parameter("out", [N, D], dt.float32, isOutput=True)
    dbg = {}
    if debug:
        for nm, shp in (
            ("dbg_xT", [P, DC * N]),
            ("dbg_keepT", [P, NT * N]),
            ("dbg_qT", [P, HP * N]),
            ("dbg_kT", [P, HP * N]),
            ("dbg_v", [P, NT * H * (DK + 1)]),
            ("dbg_hT", [DK, H * N]),
            ("dbg_p00", [P, N]),
        ):
            dbg[nm] = nc.declare_dram_parameter(nm, shp, dt.bfloat16, isOutput=True)

    with TileContext(nc) as tc, ExitStack() as ctx:
        persist = ctx.enter_context(tc.tile_pool(name="persist", bufs=1))
        stage = ctx.enter_context(tc.tile_pool(name="stage", bufs=1))
        stage_w = ctx.enter_context(tc.tile_pool(name="stage_w", bufs=2))
        expp = ctx.enter_context(tc.tile_pool(name="expp", bufs=3))
        pp = ctx.enter_context(tc.tile_pool(name="pp", bufs=6))
        recp = ctx.enter_context(tc.tile_pool(name="recp", bufs=1))
        dramp = ctx.enter_context(tc.tile_pool(name="dramp", bufs=2, space="DRAM"))
        ps_sh = ctx.enter_context(tc.tile_pool(name="ps_sh", bufs=3, space="PSUM"))
        ps_ht = ctx.enter_context(tc.tile_pool(name="ps_ht", bufs=1, space="PSUM"))

        # ---- identity for PE transposes (via regular matmul) ----
        identbf = persist.tile([P, P], dt.bfloat16)
        make_identity(nc, identbf)

        # ---- load inputs ----
        x_f32 = stage.tile([P, NT, D], dt.float32)
        nc.sync.dma_start(out=x_f32, in_=x_d[:].rearrange("(i p) d -> p i d", p=P))

        # weight layout: [P=d%128, DC=d//128, H*DK] -> a (head-pair, d-chunk)
        # stationary slice [:, j, hp*128:(hp+1)*128] is one contiguous free dim
        mask_u8 = stage.tile([P, NT, N], dt.uint8)
        nc.gpsimd.dma_start(out=mask_u8, in_=m_d[:].rearrange("(i p) m -> p i m", p=P))

        # ---- weights: DMA f32 chunks through small staging, convert to bf16
        wq_bf = persist.tile([P, DC, H * DK], dt.bfloat16)
        wk_bf = persist.tile([P, DC, H * DK], dt.bfloat16)
        wv_bf = persist.tile([P, DC, H * DK], dt.bfloat16)
        dma_engines = [nc.scalar, nc.gpsimd]
        di = 0
        for w_bf, w_d in ((wq_bf, wq_d), (wk_bf, wk_d), (wv_bf, wv_d)):
            src = w_d[:].rearrange("h (j p) k -> j p h k", p=P)
            for j in range(DC):
                wstg = stage_w.tile([P, H, DK], dt.float32, tag="wstg")
                dma_engines[di % 2].dma_start(out=wstg, in_=src[j])
                di += 1
                nc.scalar.activation(
                    out=w_bf[:, j, :],
                    in_=wstg.rearrange("p h k -> p (h k)"),
                    func=AF.Copy,
                )
        wo_bf = persist.tile([DK, H, D], dt.bfloat16)
        wo_src = wo_d[:].rearrange("h v d -> v h d")
        for c in range(4):
            wstg2 = stage_w.tile([DK, 2, D], dt.float32, tag="wstg2")
            dma_engines[di % 2].dma_start(out=wstg2, in_=wo_src[:, 2 * c : 2 * c + 2, :])
            di += 1
            nc.scalar.activation(
                out=wo_bf[:, 2 * c : 2 * c + 2, :], in_=wstg2, func=AF.Copy
            )

        # ---- xT = x^T ----
        # Transposes are regular matmuls (lhsT=block, rhs=I): the is_transpose
        # lowering (S3_LW) only supports a single sync-wait and walrus rejects
        # Tile's two-wait instructions.
        x_bf = stage.tile([P, NT, D], dt.bfloat16)
        nc.vector.tensor_copy(out=x_bf, in_=x_f32)
        xT = persist.tile([P, DC, N], dt.bfloat16)
        for j in range(DC):
            for half in range(2):
                ps = ps_sh.tile([P, N], dt.float32, tag="ps_sh")
                for k in range(4):
                    ni = half * 4 + k
                    nc.tensor.matmul(
                        ps[:, k * P : (k + 1) * P],
                        lhsT=x_bf[:, ni, j * P : (j + 1) * P],
                        rhs=identbf,
                        start=True,
                        stop=True,
                    )
                nc.vector.tensor_copy(
                    out=xT[:, j, half * 512 : (half + 1) * 512], in_=ps[:, 0:512]
                )

        # ---- keep = 1 - mask (bf16), then keepT via PE transpose ----
        m_bf = stage.tile([P, NT, N], dt.bfloat16)
        nc.gpsimd.tensor_copy(out=m_bf, in_=mask_u8)
        keep_bf = stage.tile([P, NT, N], dt.bfloat16)
        nc.gpsimd.tensor_scalar(
            out=keep_bf,
            in0=m_bf,
            scalar1=-1.0,
            scalar2=1.0,
            op0=mybir.AluOpType.mult,
            op1=mybir.AluOpType.add,
        )
        keepT = persist.tile([P, NT, N], dt.bfloat16)
        for mi in range(NT):
            for half in range(2):
                ps = ps_sh.tile([P, N], dt.float32, tag="ps_sh")
                for k in range(4):
                    ni = half * 4 + k
                    nc.tensor.matmul(
                        ps[:, k * P : (k + 1) * P],
                        lhsT=keep_bf[:, ni, mi * P : (mi + 1) * P],
                        rhs=identbf,
                        start=True,
                        stop=True,
                    )
                nc.scalar.activation(
                    out=keepT[:, mi, half * 512 : (half + 1) * 512],
                    in_=ps[:, 0:512],
                    func=AF.Copy,
                )

        # ---- projections ----
        qT = persist.tile([P, HP, N], dt.bfloat16)
        kT = persist.tile([P, HP, N], dt.bfloat16)
        for dst, w in ((qT, wq_bf), (kT, wk_bf)):
            for hp in range(HP):
                for c in range(2):
                    ps = ps_sh.tile([P, N], dt.float32, tag="ps_sh")
                    for j in range(DC):
                        nc.tensor.matmul(
                            ps[:, c * 512 : (c + 1) * 512],
                            lhsT=w[:, j, hp * P : (hp + 1) * P],
                            rhs=xT[:, j, c * 512 : (c + 1) * 512],
                            start=(j == 0),
                            stop=(j == DC - 1),
                        )
                    nc.scalar.activation(
                        out=dst[:, hp, c * 512 : (c + 1) * 512],
                        in_=ps[:, c * 512 : (c + 1) * 512],
                        func=AF.Copy,
                    )

        # v_aug: [m-part, m-tile, head, 65]; col 64 = ones (softmax denom trick)
        v_sb = persist.tile([P, NT, H, DK + 1], dt.bfloat16)
        nc.vector.memset(v_sb[:, :, :, DK : DK + 1], 1.0)
        for i in range(NT):
            ps = ps_sh.tile([P, N], dt.float32, tag="ps_sh")
            for j in range(DC):
                # one accumulation group over the full 512-col bank: PSUM
                # start=True zeroes the whole bank, so groups must not
                # interleave within a bank
                nc.tensor.matmul(
                    ps[:, 0:512],
                    lhsT=xT[:, j, i * P : (i + 1) * P],
                    rhs=wv_bf[:, j, :],
                    start=(j == 0),
                    stop=(j == DC - 1),
                )
            nc.scalar.activation(
                out=v_sb[:, i, :, 0:DK],
                in_=ps[:, 0:512].rearrange("p (h k) -> p h k", k=DK),
                func=AF.Copy,
            )

        # ---- attention per head ----
        hT = persist.tile([DK, H, N], dt.bfloat16)
        for h in range(H):
            hp, r0 = h // 2, (h % 2) * DK
            q_h = qT[r0 : r0 + DK, hp, :]
            k_h = kT[r0 : r0 + DK, hp, :]

            ps_h = ps_ht.tile([DK + 1, N], dt.float32, tag="ps_ht")
            for mi in range(NT):
                ps_s = ps_sh.tile([P, N], dt.float32, tag="ps_sh")
                for c in range(2):
                    nc.tensor.matmul(
                        ps_s[:, c * 512 : (c + 1) * 512],
                        lhsT=k_h[:, mi * P : (mi + 1) * P],
                        rhs=q_h[:, c * 512 : (c + 1) * 512],
                        start=True,
                        stop=True,
                    )
                e_t = expp.tile([P, N], dt.bfloat16, tag="e")
                nc.scalar.activation(out=e_t, in_=ps_s, func=AF.Exp, scale=0.125)
                p_t = pp.tile([P, N], dt.bfloat16, tag="p")
                nc.vector.tensor_mul(p_t, e_t, keepT[:, mi, :])
                if debug and h == 0 and mi == 0:
                    nc.sync.dma_start(out=dbg["dbg_p00"][:], in_=p_t)
                for c in range(2):
                    nc.tensor.matmul(
                        ps_h[:, c * 512 : (c + 1) * 512],
                        lhsT=v_sb[:, mi, h, :],
                        rhs=p_t[:, c * 512 : (c + 1) * 512],
                        start=(mi == 0),
                        stop=(mi == NT - 1),
                    )

            # normalize: rows 0:64 / row 64
            # denom row -> SBUF (ACT) -> DRAM -> partition-broadcast to 64
            # rows, then reciprocal on SBUF (neither reciprocal_approx_fast
            # nor DMA can read PSUM)
            den_row = recp.tile([1, N], dt.float32, tag="drow")
            nc.scalar.activation(out=den_row, in_=ps_h[DK : DK + 1, :], func=AF.Copy)
            den_dram = dramp.tile([1, N], dt.float32, tag="rdram")
            nc.sync.dma_start(out=den_dram, in_=den_row)
            den64 = recp.tile([DK, N], dt.float32, tag="d64")
            nc.sync.dma_start(out=den64, in_=den_dram.to_broadcast((DK, N)))
            rec64 = recp.tile([DK, N], dt.float32, tag="r64")
            nc.vector.reciprocal_approx_fast(out=rec64, in_=den64)
            nc.vector.tensor_mul(hT[:, h, :], ps_h[0:DK, :], rec64)

        # ---- output projection: out[n, d] = sum_h hT_h^T @ Wo_h ----
        out_sb = persist.tile([P, NT, D], dt.float32)
        for ni in range(NT):
            ps = ps_sh.tile([P, N], dt.float32, tag="ps_sh")
            for h in range(H):
                nc.tensor.matmul(
                    ps[:, 0:512],
                    lhsT=hT[:, h, ni * P : (ni + 1) * P],
                    rhs=wo_bf[:, h, :],
                    start=(h == 0),
                    stop=(h == H - 1),
                )
            nc.scalar.activation(out=out_sb[:, ni, :], in_=ps[:, 0:512], func=AF.Copy)
            nc.sync.dma_start(
                out=o_d[:].rearrange("(i p) d -> p i d", p=P)[:, ni],
                in_=out_sb[:, ni, :],
            )

        if debug:
            for nm, t, pat in (
                ("dbg_xT", xT, "p a b -> p (a b)"),
                ("dbg_keepT", keepT, "p a b -> p (a b)"),
                ("dbg_qT", qT, "p a b -> p (a b)"),
                ("dbg_kT", kT, "p a b -> p (a b)"),
                ("dbg_v", v_sb, "p a b c -> p (a b c)"),
                ("dbg_hT", hT, "p a b -> p (a b)"),
            ):
                nc.sync.dma_start(out=dbg[nm][:], in_=t.rearrange(pat))

    nc.finalize()
    return nc


_NC_CACHE = None


def kernel(**inputs: np.ndarray) -> np.ndarray:
    global _NC_CACHE
    x = inputs["x"]
    mask = inputs["mask"]
    Wq, Wk, Wv, Wo = inputs["Wq"], inputs["Wk"], inputs["Wv"], inputs["Wo"]

    if _NC_CACHE is None:
        _NC_CACHE = build_bass()
    nc = _NC_CACHE

    in_maps = []
    for b in range(B):
        in_maps.append(
            {
                "x": np.ascontiguousarray(x[b], dtype=np.float32),
                "mask": np.ascontiguousarray(mask[b]).astype(np.uint8),
                "wq": np.ascontiguousarray(Wq, dtype=np.float32),
                "wk": np.ascontiguousarray(Wk, dtype=np.float32),
                "wv": np.ascontiguousarray(Wv, dtype=np.float32),
                "wo": np.ascontiguousarray(Wo, dtype=np.float32),
            }
        )

    res = run_bass_kernel_spmd(nc, in_maps, core_ids=list(range(B)))
    out = np.stack([np.asarray(res.results[b]["out"]) for b in range(B)], axis=0)
    return out.astype(np.float32)


if __name__ == "__main__":
    rng = np.random.default_rng(0)
    ins = {
        "x": rng.standard_normal((B, N, D), dtype=np.float32),
        "mask": rng.integers(0, 2, (B, N, N)).astype(bool),
        "Wq": (rng.standard_normal((H, D, DK)) * 0.001).astype(np.float32),
        "Wk": (rng.standard_normal((H, D, DK)) * 0.001).astype(np.float32),
        "Wv": (rng.standard_normal((H, D, DK)) * 0.001).astype(np.float32),
        "Wo": (rng.standard_normal((H, DK, D)) * 0.001).astype(np.float32),
    }
    o = kernel(**ins)
    print(o.shape, o.dtype, np.abs(o).mean())
